# revision 44
# baseline (speedup 1.0000x reference)
"""AttnBlock (GroupNorm + single-head self-attention + residual) on 8 NeuronCores.

Sharding: data-parallel over B (4 batches) x sequence-parallel over query
rows (2 halves of H*W=4096) = 8 shards, one per core.  Each core loads its
batch's full x[b] as [C=128, HW=4096] fp16 (channels on partitions), with
the spatial columns rotated so the core's query half is cols [0:2048).

GroupNorm is folded into the projections: h = A*x + B (per-channel affine),
so only [128,128] weight scalings depend on the statistics.  The K-side
bias is dropped entirely: it shifts every score in a softmax row by the
same per-query constant, which softmax is invariant to.  The V bias folds
into a constant output bias (softmax rows sum to 1).  norm_w and the
q/k/v weights are negated host-side so the affine chain needs no extra
negation ops (STT has subtract but not reverse-subtract).

Main loop (32 key blocks x 2 query halves of 1024): scores transposed
St[j,i] = kT^T qT into fp32 PSUM, exp with fused bias -8 (scores bounded
~[-15,14]), PV accumulates oT[c,i] over key blocks with V stationary.
The exp work is split: most tiles on ScalarE (table exp), the first half
of every 5th key block on VectorE via a Schraudolph bit-trick building
fp16 bit patterns directly (bits = s*1477.32 + const, clamp at 0,
bitcast int16->fp16; ~3% max relative error on ~10%% of attention
weights, washed out by the softmax normalization and V-average far
below the accuracy gate).

Schedule: PE warmup matmuls gated on DMA chunk 2 lift the HAM clock-gate
to 2.4 GHz during the stats tail; projections use their own 2-slot PSUM
ring (released before the oT accumulator pool is allocated) and
interleave with the scores/exp of key blocks 0..7, so ScalarE starts
its exp stream ~24us in and stays ~97%% busy to the end of the loop.
In the loop the scores run 8 key blocks ahead of PV; the PV backlog
drains with a second group every third iteration so the last PV lands
right behind the last exp.  All PSUM evacuations run on VectorE;
ScalarE does nothing but exp (plus two epilogue copies).

Epilogue, per-half pipelined: denominators from the fp16 running e-sum
via ones-matmuls -> VectorE reciprocal -> PE transpose -> selector-
matmul broadcast; the per-query normalization fuses into the oT PSUM
evacuation (onrm = oT * r), then output projection and a fused
residual-add (STT) straight from PSUM, DMA per 512 columns.
"""

import numpy as np

C = 128
HW = 4096
NQ = 2048
HALF = 1024
JB = 32
EXP_BIAS = -8.0
EPS = 1e-5
N_CORES = 8

# Schraudolph fp16 exp: bits = round(v * 1024/ln2 + 15360 + DELTA), v = s + EXP_BIAS
SCH_A = 1024.0 / float(np.log(2.0))
SCH_DELTA = -44.2
SCH_B = 15360.0 + SCH_DELTA + EXP_BIAS * SCH_A

# wpack (f16) column offsets
_WQ, _WK, _WV, _WO = 0, 128, 256, 384
_ONESC = 512
_WPACK_W = 513
# fpack (f32) column offsets
_NW, _NB, _IDENT = 0, 1, 2
_GMAP, _GMAPT = 130, 162
_FPACK_W = 162 + 128

_NC = None


def _dve_half(jb, half):
    """Which (jb, half) exp tiles run on VectorE (Schraudolph).  Affordable
    because half the e-sum accumulation moves to the idle GpSimd engine."""
    return jb >= 2 and jb % 3 == 2 and half == 0


def _gps_esum(jb, half):
    """Which (jb, half) e-sum adds run on GpSimd (separate accumulator)."""
    return half == 1 and jb % 2 == 0


def _pin_activation_tables():
    """Restrict the table-load chooser to natural_log_exp_and_others so the
    kernel's ACT stream (ln/exp) needs a single table load."""
    from concourse.hw_specs import get_activation_tables
    tabs = get_activation_tables("gen3")
    for name in list(tabs.keys()):
        if name != "natural_log_exp_and_others":
            tabs[name] = set()


def _build_program():
    import concourse.bacc as bacc
    import concourse.tile as tile
    from concourse import mybir

    f32 = mybir.dt.float32
    f16 = mybir.dt.float16
    i16 = mybir.dt.int16
    AF = mybir.ActivationFunctionType
    OP = mybir.AluOpType

    nc = bacc.Bacc("TRN2", target_bir_lowering=False, debug=False,
                   num_devices=N_CORES)
    try:
        _pin_activation_tables()
    except Exception:
        pass

    x_d = nc.declare_dram_parameter("x", [C, HW], f16, isOutput=False)
    wpack_d = nc.declare_dram_parameter("wpack", [C, _WPACK_W], f16,
                                        isOutput=False)
    fpack_d = nc.declare_dram_parameter("fpack", [C, _FPACK_W], f32,
                                        isOutput=False)
    y_d = nc.declare_dram_parameter("y", [C, NQ], f16, isOutput=True)

    with tile.TileContext(nc) as tc:
        consts = tc.alloc_tile_pool(name="consts", bufs=1)
        big = tc.alloc_tile_pool(name="big", bufs=1)
        work = tc.alloc_tile_pool(name="work", bufs=2)
        epool = tc.alloc_tile_pool(name="epool", bufs=3)
        ypool = tc.alloc_tile_pool(name="ypool", bufs=4)
        # PSUM: pst = 2 x [128,1024]f32 score slots (4 banks); pproj = 2 x
        # [128,1024]f32 projection slots (4 banks), released before the oT
        # accumulator pool (pot, 4 banks) is allocated
        pst = tc.alloc_tile_pool(name="pst", bufs=2, space="PSUM")
        pproj = tc.alloc_tile_pool(name="pproj", bufs=2, space="PSUM")

        # ---- input DMA: x in 4 chunks across both HWDGE rings
        x16 = big.tile([C, HW], f16)
        for ch in range(4):
            eng = nc.sync if ch % 2 == 0 else nc.scalar
            eng.dma_start(out=x16[:, ch * 1024:(ch + 1) * 1024],
                          in_=x_d.ap()[:, ch * 1024:(ch + 1) * 1024])
        wpack_sb = consts.tile([C, _WPACK_W], f16)
        nc.gpsimd.dma_start(out=wpack_sb, in_=wpack_d.ap())
        fpack_sb = consts.tile([C, _FPACK_W], f32)
        nc.gpsimd.dma_start(out=fpack_sb, in_=fpack_d.ap())
        wqn_sb = wpack_sb[:, _WQ:_WQ + C]      # negated, pre-scaled by C^-0.5
        wkn_sb = wpack_sb[:, _WK:_WK + C]      # negated
        wvn_sb = wpack_sb[:, _WV:_WV + C]      # negated
        wo_sb = wpack_sb[:, _WO:_WO + C]       # positive
        onesc_sb = wpack_sb[:, _ONESC:_ONESC + 1]
        nwneg_sb = fpack_sb[:, _NW:_NW + 1]    # -norm_w
        nb_sb = fpack_sb[:, _NB:_NB + 1]
        ident_sb = fpack_sb[:, _IDENT:_IDENT + C]
        gmap_sb = fpack_sb[:, _GMAP:_GMAP + 32]  # f32, entries 0.25
        gmapt_sb = fpack_sb[0:32, _GMAPT:_GMAPT + C]  # f32

        # on-device constants (no deps, run behind the DMA)
        eps_sb = consts.tile([32, 1], f32)
        nc.vector.memset(eps_sb, EPS)
        ebias_sb = consts.tile([C, 1], f32)
        nc.vector.memset(ebias_sb, EXP_BIAS)
        # sel16[p, b*128+j] = (p == b): selector rows for the r broadcast,
        # built on GpSimd (engines cannot memset at partition offsets)
        ones16 = consts.tile([16, 16 * C], f16)
        nc.vector.memset(ones16, 1.0)
        sel16 = consts.tile([16, 16 * C], f16)
        nc.gpsimd.affine_select(
            out=sel16, in_=ones16, pattern=[[-1, 16], [0, C]],
            compare_op=OP.is_equal, fill=0.0, base=0, channel_multiplier=1)

        # ---- GroupNorm stats: per-channel mean/E[x2], combine 4ch/group via PE
        stats = work.tile([C, 8, 6], f32)
        for ch in range(8):
            nc.vector.bn_stats(out=stats[:, ch, :],
                               in_=x16[:, ch * 512:(ch + 1) * 512])
        mv = work.tile([C, 2], f32)
        nc.vector.bn_aggr(out=mv, in_=stats)
        # mv becomes (mean, E[x^2]) in place: col1 = mean^2 + var
        nc.vector.scalar_tensor_tensor(
            out=mv[:, 1:2], in0=mv[:, 0:1], scalar=mv[:, 0:1],
            in1=mv[:, 1:2], op0=OP.mult, op1=OP.add)
        # PE warmup: dummy matmuls gated on DMA chunk 2, so they run ~3us
        # of PE busy-time concurrently with the stats tail and the HAM
        # clock-gate reaches 2.4 GHz right as the projections start.
        # Emitted ahead of the tiny stats matmuls (PE runs in order); they
        # finish around the time the mean/var chain produces cstat.
        wz = consts.tile([C, 512], f16)
        nc.vector.memset(wz, 0.0)
        warm_ps = pproj.tile([C, 1024], f32, tag="pj")
        for _ in range(7):
            nc.tensor.matmul(out=warm_ps[:, 0:512], lhsT=wz[:, 0:C],
                             rhs=x16[:, 2048:2560])
        gsum = pst.tile([32, 2], f32, tag="ps")
        nc.tensor.matmul(out=gsum, lhsT=gmap_sb, rhs=mv)  # (gmean, gex2)
        gmrs = work.tile([32, 2], f32)
        nc.vector.tensor_copy(out=gmrs[:, 0:1], in_=gsum[:, 0:1])
        # nvar = gmean^2 - gex2  (negated variance, fixed by Ln scale=-1)
        nvar = work.tile([32, 1], f32)
        nc.vector.scalar_tensor_tensor(
            out=nvar, in0=gmrs[:, 0:1], scalar=gmrs[:, 0:1], in1=gsum[:, 1:2],
            op0=OP.mult, op1=OP.subtract)
        gln = work.tile([32, 1], f32)
        nc.scalar.activation(out=gln, in_=nvar, func=AF.Ln, bias=eps_sb,
                             scale=-1.0)
        nc.scalar.activation(out=gmrs[:, 1:2], in_=gln, func=AF.Exp,
                             scale=-0.5)
        cstat = pst.tile([C, 2], f32, tag="ps")
        nc.tensor.matmul(out=cstat, lhsT=gmapt_sb, rhs=gmrs)  # (mean_c, rstd_c)
        # affAneg = rstd_c * (-norm_w) = -A
        affAneg = work.tile([C, 1], f32)
        nc.vector.tensor_mul(out=affAneg, in0=cstat[:, 1:2], in1=nwneg_sb)
        # B = mean_c * (-A) + norm_b
        affB = work.tile([C, 1], f32)
        nc.vector.scalar_tensor_tensor(
            out=affB, in0=cstat[:, 0:1], scalar=affAneg, in1=nb_sb,
            op0=OP.mult, op1=OP.add)
        b16 = work.tile([C, 1], f16)
        nc.vector.tensor_copy(out=b16, in_=affB)

        # fold affine scale into projection weights ((-W)*(-A) = W*A)
        wqa = consts.tile([C, C], f16)
        nc.vector.tensor_scalar_mul(out=wqa, in0=wqn_sb, scalar1=affAneg)
        wka = consts.tile([C, C], f16)
        nc.vector.tensor_scalar_mul(out=wka, in0=wkn_sb, scalar1=affAneg)
        wva = consts.tile([C, C], f16)
        nc.vector.tensor_scalar_mul(out=wva, in0=wvn_sb, scalar1=affAneg)
        # biases: qbneg = -Wq^T B (no K bias needed), vbneg, obneg = -Wo^T Wv^T B
        pb = pst.tile([C, 2], f32, tag="ps")
        nc.tensor.matmul(out=pb[:, 0:1], lhsT=wqn_sb, rhs=b16)
        nc.tensor.matmul(out=pb[:, 1:2], lhsT=wvn_sb, rhs=b16)
        qbneg = work.tile([C, 1], f32)
        nc.vector.tensor_copy(out=qbneg, in_=pb[:, 0:1])
        vbneg16 = work.tile([C, 1], f16)
        nc.vector.tensor_copy(out=vbneg16, in_=pb[:, 1:2])
        pob = pst.tile([C, 1], f32, tag="ps")
        nc.tensor.matmul(out=pob, lhsT=wo_sb, rhs=vbneg16)
        obneg = work.tile([C, 1], f32)
        nc.vector.tensor_copy(out=obneg, in_=pob)

        qT = big.tile([C, NQ], f16)
        kT = big.tile([C, HW], f16)
        v_sb = big.tile([C, HW], f16)  # col block jb holds V[j, c] rows
        es = big.tile([C, NQ], f16)    # running exp-sum accumulator
        esG = big.tile([C, HALF], f16)  # GpSimd-side accumulator (half 1)

        def q_chunk(t):
            ps = pproj.tile([C, 1024], f32, tag="pj", name=f"qps{t}")
            for k in range(2):
                nc.tensor.matmul(out=ps[:, k * 512:(k + 1) * 512], lhsT=wqa,
                                 rhs=x16[:, t * 1024 + k * 512:
                                         t * 1024 + (k + 1) * 512])
            nc.vector.tensor_scalar_sub(out=qT[:, t * 1024:(t + 1) * 1024],
                                        in0=ps, scalar1=qbneg)

        def k_chunk(t, split=False):
            ps = pproj.tile([C, 1024], f32, tag="pj", name=f"kps{t}")
            for k in range(2):
                nc.tensor.matmul(out=ps[:, k * 512:(k + 1) * 512], lhsT=wka,
                                 rhs=x16[:, t * 1024 + k * 512:
                                         t * 1024 + (k + 1) * 512])
            if split:
                # evacuate in two pieces so the first score matmuls (which
                # only need key block 0) start one 512-copy sooner
                nc.vector.tensor_copy(out=kT[:, t * 1024:t * 1024 + 512],
                                      in_=ps[:, 0:512])
                nc.vector.tensor_copy(
                    out=kT[:, t * 1024 + 512:(t + 1) * 1024],
                    in_=ps[:, 512:1024])
            else:
                nc.vector.tensor_copy(out=kT[:, t * 1024:(t + 1) * 1024],
                                      in_=ps)

        def v_chunk(t):
            ps = pproj.tile([C, 1024], f32, tag="pj", name=f"vps{t}")
            for k in range(8):
                jb2 = t * 8 + k
                nc.tensor.matmul(out=ps[:, k * 128:(k + 1) * 128],
                                 lhsT=x16[:, jb2 * 128:(jb2 + 1) * 128],
                                 rhs=wva)
            nc.vector.tensor_copy(out=v_sb[:, t * 1024:(t + 1) * 1024],
                                  in_=ps)

        def emit_st_exp(jb):
            ets = []
            for half in range(2):
                st = pst.tile([C, HALF], f32, tag="ps", name=f"st{half}_{jb}")
                for k in range(2):
                    nc.tensor.matmul(
                        out=st[:, k * 512:(k + 1) * 512],
                        lhsT=kT[:, jb * 128:(jb + 1) * 128],
                        rhs=qT[:, half * HALF + k * 512:
                               half * HALF + (k + 1) * 512])
                e_t = epool.tile([C, HALF], f16, tag="e", bufs=10,
                                 name=f"e{half}_{jb}")
                if _dve_half(jb, half):
                    eint = epool.tile([C, HALF], i16, tag="ei",
                                      name=f"ei{half}_{jb}")
                    nc.vector.tensor_scalar(
                        out=eint, in0=st, scalar1=SCH_A, scalar2=SCH_B,
                        op0=OP.mult, op1=OP.add)
                    nc.vector.tensor_scalar_max(
                        out=e_t.bitcast(i16), in0=eint, scalar1=0)
                else:
                    nc.scalar.activation(out=e_t, in_=st, func=AF.Exp,
                                         bias=ebias_sb)
                ets.append(e_t)
            return ets

        # Pre-phase: q + k0, then scores/exp for jb 0..7 interleaved with the
        # remaining projections.  The score tiles use their own PSUM ring, so
        # ScalarE paces the exp stream from ~2.5us after the stats while the
        # PE fills its gaps with projection matmuls and VectorE drains the
        # projection evacuations.  (k chunk t covers key blocks 8t..8t+7.)
        q_chunk(0)
        k_chunk(0, split=True)
        q_chunk(1)
        ets_list = {}
        ets_list[0] = emit_st_exp(0)
        ets_list[1] = emit_st_exp(1)
        k_chunk(1)
        ets_list[2] = emit_st_exp(2)
        ets_list[3] = emit_st_exp(3)
        v_chunk(0)
        ets_list[4] = emit_st_exp(4)
        v_chunk(1)
        ets_list[5] = emit_st_exp(5)
        k_chunk(2)
        ets_list[6] = emit_st_exp(6)
        k_chunk(3)
        ets_list[7] = emit_st_exp(7)
        v_chunk(2)
        v_chunk(3)
        pproj.release()
        pot = tc.alloc_tile_pool(name="pot", bufs=1, space="PSUM")

        def emit_pv(jb):
            for half in range(2):
                for k in range(2):
                    nc.tensor.matmul(
                        out=oT[:, half * HALF + k * 512:
                               half * HALF + (k + 1) * 512],
                        lhsT=v_sb[:, jb * 128:(jb + 1) * 128],
                        rhs=ets_list[jb][half][:, k * 512:(k + 1) * 512],
                        start=(jb == 0), stop=(jb == JB - 1))
            for half in range(2):
                e_t = ets_list[jb][half]
                if _gps_esum(jb, half):
                    if jb == 0:
                        nc.gpsimd.tensor_copy(out=esG, in_=e_t)
                    else:
                        nc.gpsimd.tensor_add(out=esG, in0=esG, in1=e_t)
                else:
                    dst = es[:, half * HALF:(half + 1) * HALF]
                    first = jb == 0 if half == 0 else jb == 1
                    if first:
                        nc.vector.tensor_copy(out=dst, in_=e_t)
                    else:
                        nc.vector.tensor_add(out=dst, in0=dst, in1=e_t)
            del ets_list[jb]

        # Steady loop, software-pipelined: scores run 8 key blocks ahead of
        # PV so ScalarE never waits on the PE; the PV backlog drains with two
        # groups on every third iteration so the last PV lands right behind
        # the last exp instead of 7 groups after it.
        oT = pot.tile([C, NQ], f32, tag="ot")
        pv_next = 0
        for jb in range(JB):
            if jb + 8 < JB:
                ets_list[jb + 8] = emit_st_exp(jb + 8)
            n_pv = 2 if (jb % 3 == 2 and jb < 24) else 1
            for _ in range(n_pv):
                if pv_next < JB and pv_next <= jb + 7:
                    emit_pv(pv_next)
                    pv_next += 1
        while pv_next < JB:
            emit_pv(pv_next)
            pv_next += 1

        # ---- epilogue, per-half pipelined: denominator chain (ones-matmuls
        # -> recip -> PE transpose -> selector-matmul broadcast -> SBUF copy)
        # feeds a normalization fused into the oT evacuation (onrm = oT * r,
        # straight from PSUM), then the output projection and residual.
        # Half 1 trails half 0 by ~1us through every engine.
        # merge the GpSimd-side accumulator before the half-1 reduction
        nc.vector.tensor_add(out=es[:, HALF:NQ], in0=es[:, HALF:NQ], in1=esG)
        scols, r16s, rcs, onrms = [], [], [], []
        for half in range(2):
            scol = pst.tile([C, 8], f32, tag="ps", name=f"scol{half}")
            for m in range(8):
                ib = half * 8 + m
                nc.tensor.matmul(out=scol[:, m:m + 1],
                                 lhsT=es[:, ib * 128:(ib + 1) * 128],
                                 rhs=onesc_sb)
            scols.append(scol)
        for half in range(2):
            r_col = work.tile([C, 8], f32, name=f"rcol{half}")
            nc.vector.reciprocal(out=r_col, in_=scols[half])
            r16_ps = pst.tile([8, C], f32, tag="ps", name=f"r16ps{half}")
            nc.tensor.transpose(out=r16_ps, in_=r_col, identity=ident_sb)
            r16 = work.tile([8, C], f16, name=f"r16_{half}")
            nc.vector.tensor_copy(out=r16, in_=r16_ps)
            r16s.append(r16)
        for half in range(2):
            rbc = pst.tile([C, HALF], f32, tag="ps", name=f"rbc{half}")
            for m in range(8):
                nc.tensor.matmul(out=rbc[:, m * 128:(m + 1) * 128],
                                 lhsT=sel16[0:8, m * C:(m + 1) * C],
                                 rhs=r16s[half])
            rc_sb = work.tile([C, HALF], f16, name=f"rc{half}")
            nc.scalar.copy(out=rc_sb, in_=rbc)
            rcs.append(rc_sb)
            onrm = big.tile([C, HALF], f16, name=f"onrm{half}")
            nc.vector.tensor_mul(
                out=onrm, in0=oT[:, half * HALF:(half + 1) * HALF],
                in1=rc_sb)
            onrms.append(onrm)
        for half in range(2):
            op_ps = pst.tile([C, HALF], f32, tag="ps", name=f"op{half}")
            for k in range(2):
                nc.tensor.matmul(out=op_ps[:, k * 512:(k + 1) * 512],
                                 lhsT=wo_sb,
                                 rhs=onrms[half][:, k * 512:(k + 1) * 512])
            for k in range(2):
                i0 = half * HALF + k * 512
                y_sb = ypool.tile([C, 512], f16, name=f"y{half}_{k}")
                nc.vector.scalar_tensor_tensor(
                    out=y_sb, in0=op_ps[:, k * 512:(k + 1) * 512],
                    scalar=obneg,
                    in1=x16[:, i0:i0 + 512],
                    op0=OP.subtract, op1=OP.add)
                eng = nc.sync if k % 2 == 0 else nc.scalar
                eng.dma_start(out=y_d.ap()[:, i0:i0 + 512], in_=y_sb)

        for p in (pot, pst, ypool, epool, work, big, consts):
            p.release()

    nc.compile()
    return nc


def _get_nc():
    global _NC
    if _NC is None:
        _NC = _build_program()
    return _NC


def _make_packs(inputs):
    wq = -(np.asarray(inputs["Wq"], dtype=np.float32) * (C ** -0.5)).astype(np.float16)
    wk = -np.asarray(inputs["Wk"], dtype=np.float32).astype(np.float16)
    wv = -np.asarray(inputs["Wv"], dtype=np.float32).astype(np.float16)
    wo = np.asarray(inputs["Wo"], dtype=np.float32).astype(np.float16)
    gmap = np.zeros((C, 32), np.float32)
    for c in range(C):
        gmap[c, c // 4] = 0.25
    wpack = np.zeros((C, _WPACK_W), np.float16)
    wpack[:, _WQ:_WQ + C] = wq
    wpack[:, _WK:_WK + C] = wk
    wpack[:, _WV:_WV + C] = wv
    wpack[:, _WO:_WO + C] = wo
    wpack[:, _ONESC:_ONESC + 1] = 1.0
    fpack = np.zeros((C, _FPACK_W), np.float32)
    fpack[:, _NW] = -np.asarray(inputs["norm_w"], dtype=np.float32)
    fpack[:, _NB] = np.asarray(inputs["norm_b"], dtype=np.float32)
    fpack[:, _IDENT:_IDENT + C] = np.eye(C, dtype=np.float32)
    fpack[:, _GMAP:_GMAP + 32] = gmap
    fpack[0:32, _GMAPT:_GMAPT + C] = np.sign(gmap.T)
    return wpack, fpack


def _make_in_maps(inputs):
    x = np.asarray(inputs["x"], dtype=np.float32).astype(np.float16)
    B = x.shape[0]
    xf = x.reshape(B, C, HW)
    wpack, fpack = _make_packs(inputs)
    in_maps = []
    for core in range(N_CORES):
        b, s = core // 2, core % 2
        xb = xf[b]
        if s == 1:
            xb = np.concatenate([xb[:, NQ:], xb[:, :NQ]], axis=1)
        in_maps.append({
            "x": np.ascontiguousarray(xb),
            "wpack": wpack, "fpack": fpack,
        })
    return in_maps


def kernel(**inputs):
    from concourse.bass_utils import run_bass_kernel_spmd

    nc = _get_nc()
    in_maps = _make_in_maps(inputs)
    res = run_bass_kernel_spmd(nc, in_maps, list(range(N_CORES)))
    x = np.asarray(inputs["x"], dtype=np.float32)
    B, _, H, W = x.shape
    out = np.empty((B, C, HW), np.float32)
    for core in range(N_CORES):
        b, s = core // 2, core % 2
        out[b, :, s * NQ:(s + 1) * NQ] = res.results[core]["y"].astype(np.float32)
    return out.reshape(B, C, H, W)


# revision 46
# speedup vs baseline: 1.2569x; 1.2569x over previous
"""AttnBlock (GroupNorm + single-head self-attention + residual) on 8 NeuronCores.

Sharding: data-parallel over B (4 batches) x sequence-parallel over query
rows (2 halves of H*W=4096) = 8 shards, one per core.  Each core loads its
batch's full x[b] as [C=128, HW=4096] fp16 (channels on partitions), with
the spatial columns rotated so the core's query half is cols [0:2048).

GroupNorm is folded into the projections: h = A*x + B (per-channel affine),
so only [128,128] weight scalings depend on the statistics.  The K-side
bias is dropped entirely: it shifts every score in a softmax row by the
same per-query constant, which softmax is invariant to.  The V bias folds
into a constant output bias (softmax rows sum to 1).  norm_w and the
q/k/v weights are negated host-side so the affine chain needs no extra
negation ops (STT has subtract but not reverse-subtract).

Main loop (32 key blocks x 2 query halves of 1024): scores transposed
St[j,i] = kT^T qT into fp32 PSUM, exp with fused bias -8 (scores bounded
~[-15,14]), PV accumulates oT[c,i] over key blocks with V stationary.
The exp work is split: most tiles on ScalarE (table exp), the first half
of every 5th key block on VectorE via a Schraudolph bit-trick building
fp16 bit patterns directly (bits = s*1477.32 + const, clamp at 0,
bitcast int16->fp16; ~3% max relative error on ~10%% of attention
weights, washed out by the softmax normalization and V-average far
below the accuracy gate).

Schedule: PE warmup matmuls gated on DMA chunk 2 lift the HAM clock-gate
to 2.4 GHz during the stats tail; projections use their own 2-slot PSUM
ring (released before the oT accumulator pool is allocated) and
interleave with the scores/exp of key blocks 0..7, so ScalarE starts
its exp stream ~24us in and stays ~97%% busy to the end of the loop.
In the loop the scores run 8 key blocks ahead of PV; the PV backlog
drains with a second group every third iteration so the last PV lands
right behind the last exp.  All PSUM evacuations run on VectorE;
ScalarE does nothing but exp (plus two epilogue copies).

Epilogue, per-half pipelined: denominators from the fp16 running e-sum
via ones-matmuls -> VectorE reciprocal -> PE transpose -> selector-
matmul broadcast; the per-query normalization fuses into the oT PSUM
evacuation (onrm = oT * r), then output projection and a fused
residual-add (STT) straight from PSUM, DMA per 512 columns.
"""

import numpy as np

C = 128
HW = 4096
NQ = 2048
HALF = 1024
JB = 32
EXP_BIAS = -8.0
EPS = 1e-5
N_CORES = 8

# Schraudolph fp16 exp: bits = round(v * 1024/ln2 + 15360 + DELTA), v = s + EXP_BIAS
SCH_A = 1024.0 / float(np.log(2.0))
SCH_DELTA = -44.2
SCH_B = 15360.0 + SCH_DELTA + EXP_BIAS * SCH_A

# wpack (f16) column offsets
_WQ, _WK, _WV, _WO = 0, 128, 256, 384
_ONESC = 512
_WPACK_W = 513
# fpack (f32) column offsets
_NW, _NB, _IDENT = 0, 1, 2
_GMAP, _GMAPT = 130, 162
_FPACK_W = 162 + 128

_NC = None


def _dve_half(jb, half):
    """Which (jb, half) exp tiles run on VectorE (Schraudolph)."""
    return jb % 5 == 4 and half == 0


def _pin_activation_tables():
    """Restrict the table-load chooser to natural_log_exp_and_others so the
    kernel's ACT stream (ln/exp) needs a single table load."""
    from concourse.hw_specs import get_activation_tables
    tabs = get_activation_tables("gen3")
    for name in list(tabs.keys()):
        if name != "natural_log_exp_and_others":
            tabs[name] = set()


def _build_program():
    import concourse.bacc as bacc
    import concourse.tile as tile
    from concourse import mybir

    f32 = mybir.dt.float32
    f16 = mybir.dt.float16
    i16 = mybir.dt.int16
    AF = mybir.ActivationFunctionType
    OP = mybir.AluOpType

    nc = bacc.Bacc("TRN2", target_bir_lowering=False, debug=False,
                   num_devices=N_CORES)
    try:
        _pin_activation_tables()
    except Exception:
        pass

    x_d = nc.declare_dram_parameter("x", [C, HW], f16, isOutput=False)
    wpack_d = nc.declare_dram_parameter("wpack", [C, _WPACK_W], f16,
                                        isOutput=False)
    fpack_d = nc.declare_dram_parameter("fpack", [C, _FPACK_W], f32,
                                        isOutput=False)
    y_d = nc.declare_dram_parameter("y", [C, NQ], f16, isOutput=True)

    with tile.TileContext(nc) as tc:
        consts = tc.alloc_tile_pool(name="consts", bufs=1)
        big = tc.alloc_tile_pool(name="big", bufs=1)
        work = tc.alloc_tile_pool(name="work", bufs=2)
        epool = tc.alloc_tile_pool(name="epool", bufs=3)
        ypool = tc.alloc_tile_pool(name="ypool", bufs=4)
        # PSUM: pst = 2 x [128,1024]f32 score slots (4 banks); pproj = 2 x
        # [128,1024]f32 projection slots (4 banks), released before the oT
        # accumulator pool (pot, 4 banks) is allocated
        pst = tc.alloc_tile_pool(name="pst", bufs=2, space="PSUM")
        pproj = tc.alloc_tile_pool(name="pproj", bufs=2, space="PSUM")

        # ---- input DMA: x in 4 chunks across both HWDGE rings
        x16 = big.tile([C, HW], f16)
        for ch in range(4):
            eng = nc.sync if ch % 2 == 0 else nc.scalar
            eng.dma_start(out=x16[:, ch * 1024:(ch + 1) * 1024],
                          in_=x_d.ap()[:, ch * 1024:(ch + 1) * 1024])
        wpack_sb = consts.tile([C, _WPACK_W], f16)
        nc.gpsimd.dma_start(out=wpack_sb, in_=wpack_d.ap())
        fpack_sb = consts.tile([C, _FPACK_W], f32)
        nc.gpsimd.dma_start(out=fpack_sb, in_=fpack_d.ap())
        wqn_sb = wpack_sb[:, _WQ:_WQ + C]      # negated, pre-scaled by C^-0.5
        wkn_sb = wpack_sb[:, _WK:_WK + C]      # negated
        wvn_sb = wpack_sb[:, _WV:_WV + C]      # negated
        wo_sb = wpack_sb[:, _WO:_WO + C]       # positive
        onesc_sb = wpack_sb[:, _ONESC:_ONESC + 1]
        nwneg_sb = fpack_sb[:, _NW:_NW + 1]    # -norm_w
        nb_sb = fpack_sb[:, _NB:_NB + 1]
        ident_sb = fpack_sb[:, _IDENT:_IDENT + C]
        gmap_sb = fpack_sb[:, _GMAP:_GMAP + 32]  # f32, entries 0.25
        gmapt_sb = fpack_sb[0:32, _GMAPT:_GMAPT + C]  # f32

        # on-device constants (no deps, run behind the DMA)
        eps_sb = consts.tile([32, 1], f32)
        nc.vector.memset(eps_sb, EPS)
        ebias_sb = consts.tile([C, 1], f32)
        nc.vector.memset(ebias_sb, EXP_BIAS)
        # sel16[p, b*128+j] = (p == b): selector rows for the r broadcast,
        # built on GpSimd (engines cannot memset at partition offsets)
        ones16 = consts.tile([16, 16 * C], f16)
        nc.vector.memset(ones16, 1.0)
        sel16 = consts.tile([16, 16 * C], f16)
        nc.gpsimd.affine_select(
            out=sel16, in_=ones16, pattern=[[-1, 16], [0, C]],
            compare_op=OP.is_equal, fill=0.0, base=0, channel_multiplier=1)

        # ---- GroupNorm stats: per-channel mean/E[x2], combine 4ch/group via PE
        stats = work.tile([C, 8, 6], f32)
        for ch in range(8):
            nc.vector.bn_stats(out=stats[:, ch, :],
                               in_=x16[:, ch * 512:(ch + 1) * 512])
        mv = work.tile([C, 2], f32)
        nc.vector.bn_aggr(out=mv, in_=stats)
        # mv becomes (mean, E[x^2]) in place: col1 = mean^2 + var
        nc.vector.scalar_tensor_tensor(
            out=mv[:, 1:2], in0=mv[:, 0:1], scalar=mv[:, 0:1],
            in1=mv[:, 1:2], op0=OP.mult, op1=OP.add)
        # PE warmup: dummy matmuls gated on DMA chunk 2, so they run ~3us
        # of PE busy-time concurrently with the stats tail and the HAM
        # clock-gate reaches 2.4 GHz right as the projections start.
        # Emitted ahead of the tiny stats matmuls (PE runs in order); they
        # finish around the time the mean/var chain produces cstat.
        wz = consts.tile([C, 512], f16)
        nc.vector.memset(wz, 0.0)
        warm_ps = pproj.tile([C, 1024], f32, tag="pj")
        for _ in range(7):
            nc.tensor.matmul(out=warm_ps[:, 0:512], lhsT=wz[:, 0:C],
                             rhs=x16[:, 2048:2560])
        gsum = pst.tile([32, 2], f32, tag="ps")
        nc.tensor.matmul(out=gsum, lhsT=gmap_sb, rhs=mv)  # (gmean, gex2)
        gmrs = work.tile([32, 2], f32)
        nc.vector.tensor_copy(out=gmrs[:, 0:1], in_=gsum[:, 0:1])
        # nvar = gmean^2 - gex2  (negated variance, fixed by Ln scale=-1)
        nvar = work.tile([32, 1], f32)
        nc.vector.scalar_tensor_tensor(
            out=nvar, in0=gmrs[:, 0:1], scalar=gmrs[:, 0:1], in1=gsum[:, 1:2],
            op0=OP.mult, op1=OP.subtract)
        gln = work.tile([32, 1], f32)
        nc.scalar.activation(out=gln, in_=nvar, func=AF.Ln, bias=eps_sb,
                             scale=-1.0)
        nc.scalar.activation(out=gmrs[:, 1:2], in_=gln, func=AF.Exp,
                             scale=-0.5)
        cstat = pst.tile([C, 2], f32, tag="ps")
        nc.tensor.matmul(out=cstat, lhsT=gmapt_sb, rhs=gmrs)  # (mean_c, rstd_c)
        # affAneg = rstd_c * (-norm_w) = -A
        affAneg = work.tile([C, 1], f32)
        nc.vector.tensor_mul(out=affAneg, in0=cstat[:, 1:2], in1=nwneg_sb)
        # B = mean_c * (-A) + norm_b
        affB = work.tile([C, 1], f32)
        nc.vector.scalar_tensor_tensor(
            out=affB, in0=cstat[:, 0:1], scalar=affAneg, in1=nb_sb,
            op0=OP.mult, op1=OP.add)
        b16 = work.tile([C, 1], f16)
        nc.vector.tensor_copy(out=b16, in_=affB)

        # fold affine scale into projection weights ((-W)*(-A) = W*A)
        wqa = consts.tile([C, C], f16)
        nc.vector.tensor_scalar_mul(out=wqa, in0=wqn_sb, scalar1=affAneg)
        wka = consts.tile([C, C], f16)
        nc.vector.tensor_scalar_mul(out=wka, in0=wkn_sb, scalar1=affAneg)
        wva = consts.tile([C, C], f16)
        nc.vector.tensor_scalar_mul(out=wva, in0=wvn_sb, scalar1=affAneg)
        # biases: qbneg = -Wq^T B (no K bias needed), vbneg, obneg = -Wo^T Wv^T B
        pb = pst.tile([C, 2], f32, tag="ps")
        nc.tensor.matmul(out=pb[:, 0:1], lhsT=wqn_sb, rhs=b16)
        nc.tensor.matmul(out=pb[:, 1:2], lhsT=wvn_sb, rhs=b16)
        qbneg = work.tile([C, 1], f32)
        nc.vector.tensor_copy(out=qbneg, in_=pb[:, 0:1])
        vbneg16 = work.tile([C, 1], f16)
        nc.vector.tensor_copy(out=vbneg16, in_=pb[:, 1:2])
        pob = pst.tile([C, 1], f32, tag="ps")
        nc.tensor.matmul(out=pob, lhsT=wo_sb, rhs=vbneg16)
        obneg = work.tile([C, 1], f32)
        nc.vector.tensor_copy(out=obneg, in_=pob)

        qT = big.tile([C, NQ], f16)
        kT = big.tile([C, HW], f16)
        v_sb = big.tile([C, HW], f16)  # col block jb holds V[j, c] rows
        es = big.tile([C, NQ], f16)    # running exp-sum accumulator

        def q_chunk(t):
            ps = pproj.tile([C, 1024], f32, tag="pj", name=f"qps{t}")
            for k in range(2):
                nc.tensor.matmul(out=ps[:, k * 512:(k + 1) * 512], lhsT=wqa,
                                 rhs=x16[:, t * 1024 + k * 512:
                                         t * 1024 + (k + 1) * 512])
            if t == 0:
                # split evacuation: the first score matmul only reads
                # qT[:, 0:512], so it starts one 512-copy sooner
                for k in range(2):
                    nc.vector.tensor_scalar_sub(
                        out=qT[:, k * 512:(k + 1) * 512],
                        in0=ps[:, k * 512:(k + 1) * 512], scalar1=qbneg)
            else:
                nc.vector.tensor_scalar_sub(
                    out=qT[:, t * 1024:(t + 1) * 1024], in0=ps,
                    scalar1=qbneg)

        def k_chunk(t, split=False):
            ps = pproj.tile([C, 1024], f32, tag="pj", name=f"kps{t}")
            for k in range(2):
                nc.tensor.matmul(out=ps[:, k * 512:(k + 1) * 512], lhsT=wka,
                                 rhs=x16[:, t * 1024 + k * 512:
                                         t * 1024 + (k + 1) * 512])
            if split:
                # evacuate in two pieces so the first score matmuls (which
                # only need key block 0) start one 512-copy sooner
                nc.vector.tensor_copy(out=kT[:, t * 1024:t * 1024 + 512],
                                      in_=ps[:, 0:512])
                nc.vector.tensor_copy(
                    out=kT[:, t * 1024 + 512:(t + 1) * 1024],
                    in_=ps[:, 512:1024])
            else:
                nc.vector.tensor_copy(out=kT[:, t * 1024:(t + 1) * 1024],
                                      in_=ps)

        def v_chunk(t):
            ps = pproj.tile([C, 1024], f32, tag="pj", name=f"vps{t}")
            for k in range(8):
                jb2 = t * 8 + k
                nc.tensor.matmul(out=ps[:, k * 128:(k + 1) * 128],
                                 lhsT=x16[:, jb2 * 128:(jb2 + 1) * 128],
                                 rhs=wva)
            nc.vector.tensor_copy(out=v_sb[:, t * 1024:(t + 1) * 1024],
                                  in_=ps)

        def emit_st_exp(jb):
            # at offloaded blocks emit the ScalarE half's scores first so
            # the exp stream never waits behind the DVE half's matmuls
            ets = [None, None]
            halves = (1, 0) if _dve_half(jb, 0) else (0, 1)
            for half in halves:
                st = pst.tile([C, HALF], f32, tag="ps", name=f"st{half}_{jb}")
                for k in range(2):
                    nc.tensor.matmul(
                        out=st[:, k * 512:(k + 1) * 512],
                        lhsT=kT[:, jb * 128:(jb + 1) * 128],
                        rhs=qT[:, half * HALF + k * 512:
                               half * HALF + (k + 1) * 512])
                e_t = epool.tile([C, HALF], f16, tag="e", bufs=10,
                                 name=f"e{half}_{jb}")
                if _dve_half(jb, half):
                    eint = epool.tile([C, HALF], i16, tag="ei",
                                      name=f"ei{half}_{jb}")
                    nc.vector.tensor_scalar(
                        out=eint, in0=st, scalar1=SCH_A, scalar2=SCH_B,
                        op0=OP.mult, op1=OP.add)
                    nc.vector.tensor_scalar_max(
                        out=e_t.bitcast(i16), in0=eint, scalar1=0)
                else:
                    nc.scalar.activation(out=e_t, in_=st, func=AF.Exp,
                                         bias=ebias_sb)
                ets[half] = e_t
            return ets

        # Pre-phase: q + k0, then scores/exp for jb 0..7 interleaved with the
        # remaining projections.  The score tiles use their own PSUM ring, so
        # ScalarE paces the exp stream from ~2.5us after the stats while the
        # PE fills its gaps with projection matmuls and VectorE drains the
        # projection evacuations.  (k chunk t covers key blocks 8t..8t+7.)
        q_chunk(0)
        k_chunk(0, split=True)
        q_chunk(1)
        ets_list = {}
        ets_list[0] = emit_st_exp(0)
        ets_list[1] = emit_st_exp(1)
        k_chunk(1)
        ets_list[2] = emit_st_exp(2)
        ets_list[3] = emit_st_exp(3)
        v_chunk(0)
        ets_list[4] = emit_st_exp(4)
        v_chunk(1)
        ets_list[5] = emit_st_exp(5)
        k_chunk(2)
        ets_list[6] = emit_st_exp(6)
        k_chunk(3)
        ets_list[7] = emit_st_exp(7)
        v_chunk(2)
        v_chunk(3)
        pproj.release()
        pot = tc.alloc_tile_pool(name="pot", bufs=1, space="PSUM")

        def emit_pv(jb):
            for half in range(2):
                for k in range(2):
                    nc.tensor.matmul(
                        out=oT[:, half * HALF + k * 512:
                               half * HALF + (k + 1) * 512],
                        lhsT=v_sb[:, jb * 128:(jb + 1) * 128],
                        rhs=ets_list[jb][half][:, k * 512:(k + 1) * 512],
                        start=(jb == 0), stop=(jb == JB - 1))
            for half in range(2):
                dst = es[:, half * HALF:(half + 1) * HALF]
                if jb == 0:
                    nc.vector.tensor_copy(out=dst, in_=ets_list[jb][half])
                else:
                    nc.vector.tensor_add(out=dst, in0=dst,
                                         in1=ets_list[jb][half])
            del ets_list[jb]

        # Steady loop, software-pipelined: scores run 8 key blocks ahead of
        # PV so ScalarE never waits on the PE; the PV backlog drains with two
        # groups on every third iteration so the last PV lands right behind
        # the last exp instead of 7 groups after it.
        oT = pot.tile([C, NQ], f32, tag="ot")
        pv_next = 0
        for jb in range(JB):
            if jb + 8 < JB:
                ets_list[jb + 8] = emit_st_exp(jb + 8)
            n_pv = 2 if (jb % 3 == 2 and jb < 24) else 1
            for _ in range(n_pv):
                if pv_next < JB and pv_next <= jb + 7:
                    emit_pv(pv_next)
                    pv_next += 1
        while pv_next < JB:
            emit_pv(pv_next)
            pv_next += 1

        # ---- epilogue, per-half pipelined: denominator chain (ones-matmuls
        # -> recip -> PE transpose -> selector-matmul broadcast -> SBUF copy)
        # feeds a normalization fused into the oT evacuation (onrm = oT * r,
        # straight from PSUM), then the output projection and residual.
        # Half 1 trails half 0 by ~1us through every engine.
        scols, r16s, rcs, onrms = [], [], [], []
        for half in range(2):
            scol = pst.tile([C, 8], f32, tag="ps", name=f"scol{half}")
            for m in range(8):
                ib = half * 8 + m
                nc.tensor.matmul(out=scol[:, m:m + 1],
                                 lhsT=es[:, ib * 128:(ib + 1) * 128],
                                 rhs=onesc_sb)
            scols.append(scol)
        for half in range(2):
            r_col = work.tile([C, 8], f32, name=f"rcol{half}")
            nc.vector.reciprocal(out=r_col, in_=scols[half])
            r16_ps = pst.tile([8, C], f32, tag="ps", name=f"r16ps{half}")
            nc.tensor.transpose(out=r16_ps, in_=r_col, identity=ident_sb)
            r16 = work.tile([8, C], f16, name=f"r16_{half}")
            nc.vector.tensor_copy(out=r16, in_=r16_ps)
            r16s.append(r16)
        for half in range(2):
            rbc = pst.tile([C, HALF], f32, tag="ps", name=f"rbc{half}")
            for m in range(8):
                nc.tensor.matmul(out=rbc[:, m * 128:(m + 1) * 128],
                                 lhsT=sel16[0:8, m * C:(m + 1) * C],
                                 rhs=r16s[half])
            rc_sb = work.tile([C, HALF], f16, name=f"rc{half}")
            nc.scalar.copy(out=rc_sb, in_=rbc)
            rcs.append(rc_sb)
            onrm = big.tile([C, HALF], f16, name=f"onrm{half}")
            nc.vector.tensor_mul(
                out=onrm, in0=oT[:, half * HALF:(half + 1) * HALF],
                in1=rc_sb)
            onrms.append(onrm)
        for half in range(2):
            op_ps = pst.tile([C, HALF], f32, tag="ps", name=f"op{half}")
            for k in range(2):
                nc.tensor.matmul(out=op_ps[:, k * 512:(k + 1) * 512],
                                 lhsT=wo_sb,
                                 rhs=onrms[half][:, k * 512:(k + 1) * 512])
            for k in range(2):
                i0 = half * HALF + k * 512
                y_sb = ypool.tile([C, 512], f16, name=f"y{half}_{k}")
                nc.vector.scalar_tensor_tensor(
                    out=y_sb, in0=op_ps[:, k * 512:(k + 1) * 512],
                    scalar=obneg,
                    in1=x16[:, i0:i0 + 512],
                    op0=OP.subtract, op1=OP.add)
                eng = nc.sync if k % 2 == 0 else nc.scalar
                eng.dma_start(out=y_d.ap()[:, i0:i0 + 512], in_=y_sb)

        for p in (pot, pst, ypool, epool, work, big, consts):
            p.release()

    nc.compile()
    return nc


def _get_nc():
    global _NC
    if _NC is None:
        _NC = _build_program()
    return _NC


def _make_packs(inputs):
    wq = -(np.asarray(inputs["Wq"], dtype=np.float32) * (C ** -0.5)).astype(np.float16)
    wk = -np.asarray(inputs["Wk"], dtype=np.float32).astype(np.float16)
    wv = -np.asarray(inputs["Wv"], dtype=np.float32).astype(np.float16)
    wo = np.asarray(inputs["Wo"], dtype=np.float32).astype(np.float16)
    gmap = np.zeros((C, 32), np.float32)
    for c in range(C):
        gmap[c, c // 4] = 0.25
    wpack = np.zeros((C, _WPACK_W), np.float16)
    wpack[:, _WQ:_WQ + C] = wq
    wpack[:, _WK:_WK + C] = wk
    wpack[:, _WV:_WV + C] = wv
    wpack[:, _WO:_WO + C] = wo
    wpack[:, _ONESC:_ONESC + 1] = 1.0
    fpack = np.zeros((C, _FPACK_W), np.float32)
    fpack[:, _NW] = -np.asarray(inputs["norm_w"], dtype=np.float32)
    fpack[:, _NB] = np.asarray(inputs["norm_b"], dtype=np.float32)
    fpack[:, _IDENT:_IDENT + C] = np.eye(C, dtype=np.float32)
    fpack[:, _GMAP:_GMAP + 32] = gmap
    fpack[0:32, _GMAPT:_GMAPT + C] = np.sign(gmap.T)
    return wpack, fpack


def _make_in_maps(inputs):
    x = np.asarray(inputs["x"], dtype=np.float32).astype(np.float16)
    B = x.shape[0]
    xf = x.reshape(B, C, HW)
    wpack, fpack = _make_packs(inputs)
    in_maps = []
    for core in range(N_CORES):
        b, s = core // 2, core % 2
        xb = xf[b]
        if s == 1:
            xb = np.concatenate([xb[:, NQ:], xb[:, :NQ]], axis=1)
        in_maps.append({
            "x": np.ascontiguousarray(xb),
            "wpack": wpack, "fpack": fpack,
        })
    return in_maps


def kernel(**inputs):
    from concourse.bass_utils import run_bass_kernel_spmd

    nc = _get_nc()
    in_maps = _make_in_maps(inputs)
    res = run_bass_kernel_spmd(nc, in_maps, list(range(N_CORES)))
    x = np.asarray(inputs["x"], dtype=np.float32)
    B, _, H, W = x.shape
    out = np.empty((B, C, HW), np.float32)
    for core in range(N_CORES):
        b, s = core // 2, core % 2
        out[b, :, s * NQ:(s + 1) * NQ] = res.results[core]["y"].astype(np.float32)
    return out.reshape(B, C, H, W)
